# revision 7
# baseline (speedup 1.0000x reference)
"""Trainium2 Bass kernel for nn_Aggregator (gnn_message_passing).

pooled[B,D] = owner_masks.f32 @ ((nodes@Wt + bt) * sigmoid(nodes@Wg + bg))

Sharding: nodes (and owner_masks columns) split along N across 8 cores.
Each core computes a partial [B, 2D] = [M@(A*G) | M@G]; the host sums the
8 partials and applies the bt column bias algebraically:
    pooled = sum_c pool1_c + (sum_c pool2_c) * bt[None, :]
(exact: (A + 1 bt^T) * G = A*G + (1 bt^T)*G and M @ ((1 bt^T)*G) =
(M@G) diag(bt)).

Device pipeline (per core; fp16 inputs, fp32 accumulation):
  nodesT  [S=128, NPAD]   node features transposed, S on partitions
  masksT  [128, NT, B]    partition-major transposed masks:
                          masksT[p, t, b] = owner_masks[b, t*128+p]
  per chunk of 4 128-node tiles:
    PE : MM(psum_d[:,sl], lhsT=nodesT[:,sl], rhs=Wt)  (A tile, [n,D])
         MM(psum_g[:,sl], lhsT=nodesT[:,sl], rhs=Wg)  (G_pre,  [n,D])
    DVE: gpre = psum_g + bg_bcast          (fp16 out)
    ACT: mg[:, :, D:]  = sigmoid(gpre)     (fp16)
    ACT: mg[:, :, :D]  = copy(psum_d)      (fp16)
    DVE: mg[:, :, :D] *= mg[:, :, D:]      (in-place msg = A*G)
    PE : pool12[B, 2D] += masksT_t.T @ mg_t   (one 256-wide MM per tile)
"""

import json

import numpy as np

import concourse.bass as bass
import concourse.mybir as mybir
import concourse.tile as tile
from concourse import bass2jax as _b2j
from concourse import bass_utils as _bu
from concourse.bass_utils import run_bass_kernel_spmd


def _split_excess_waits_json(bir_json) -> bytes:
    """Walrus in this container accepts at most 1 embedded sem-wait per
    instruction (2 for EventSemaphore). Tile emits instructions (notably the
    kernel-tail Drain) with more. Move excess waits onto injected
    EventSemaphore instructions placed immediately before the offender in
    the same engine stream — identical blocking semantics."""
    if isinstance(bir_json, str):
        bir_json = bir_json.encode()
    d = json.loads(bir_json)
    counter = [0]

    def fix_block(b):
        new = []
        for inst in b.get("instructions", []):
            si = inst.get("sync_info")
            waits = (si or {}).get("on_wait") or []
            cap = 2 if inst.get("opcode") == "EventSemaphore" else 1
            if len(waits) > cap:
                keep, excess = waits[:cap], waits[cap:]
                for j in range(0, len(excess), 2):
                    counter[0] += 1
                    new.append(
                        {
                            "debug": inst.get("debug"),
                            "engine": inst["engine"],
                            "ins": [],
                            "outs": [],
                            "name": f"antsplit_ev_{counter[0]}",
                            "opcode": "EventSemaphore",
                            "sync_info": {
                                "on_update": [],
                                "on_wait": excess[j : j + 2],
                            },
                        }
                    )
                si["on_wait"] = keep
            new.append(inst)
        b["instructions"] = new
        for sb in b.get("blocks", []):
            fix_block(sb)

    for f in d.get("functions", []):
        for blk in f.get("blocks", []):
            fix_block(blk)
    return json.dumps(d).encode()


if not getattr(_bu, "_ant_split_waits_patched", False):
    _orig_compile_bir_kernel = _bu.compile_bir_kernel

    def _patched_compile_bir_kernel(bir_json, tmpdir, neff_name="file.neff"):
        return _orig_compile_bir_kernel(
            _split_excess_waits_json(bir_json), tmpdir, neff_name
        )

    _bu.compile_bir_kernel = _patched_compile_bir_kernel
    _b2j.compile_bir_kernel = _patched_compile_bir_kernel
    _bu._ant_split_waits_patched = True

N_CORES = 8
N_TOTAL = 500_000
B = 128
S = 128
D = 128
P = 128

N_PER_CORE = N_TOTAL // N_CORES          # 62500
TILES_PER_CHUNK = 4
CHUNK = TILES_PER_CHUNK * P              # 512
N_CHUNKS = -(-N_PER_CORE // CHUNK)       # 123
N_TILES = N_CHUNKS * TILES_PER_CHUNK     # 492
N_PAD = N_TILES * P                      # 62976

F16 = mybir.dt.float16
F32 = mybir.dt.float32
NP_F16 = np.float16


# bisect/debug switches
OPTS = {
    "touches": True,     # one-time const touch ops
    "sigmoid": True,     # False -> plain Copy instead of Sigmoid
    "mm2": True,         # False -> skip the pooling matmuls (evict psum_d instead)
    "mm1": True,         # False -> skip the feature matmuls (memset psums... keep)
}


def build_bass() -> bass.Bass:
    nc = bass.Bass()

    nodesT = nc.dram_tensor("nodesT", [P, N_PAD], F16, kind="ExternalInput").ap()
    masksT = nc.dram_tensor("masksT", [P, N_TILES, B], F16, kind="ExternalInput").ap()
    wt_d = nc.dram_tensor("wt", [S, D], F16, kind="ExternalInput").ap()
    wg_d = nc.dram_tensor("wg", [S, D], F16, kind="ExternalInput").ap()
    bgb_d = nc.dram_tensor("bgb", [P, CHUNK], F32, kind="ExternalInput").ap()
    out_d = nc.dram_tensor("out", [B, 2 * D], F32, kind="ExternalOutput").ap()

    with tile.TileContext(nc) as tc:
        with (
            tc.tile_pool(name="consts", bufs=1) as consts,
            tc.tile_pool(name="scratch", bufs=1) as scratch,
            tc.tile_pool(name="nodes", bufs=4) as nodes_pool,
            tc.tile_pool(name="masks", bufs=4) as masks_pool,
            tc.tile_pool(name="gpre", bufs=3) as gpre_pool,
            tc.tile_pool(name="dt", bufs=3) as d_pool,
            tc.tile_pool(name="gt", bufs=3) as g_pool,
            tc.tile_pool(name="outs", bufs=1) as out_pool,
            tc.tile_pool(name="psd", bufs=2, space="PSUM") as psd_pool,
            tc.tile_pool(name="psg", bufs=2, space="PSUM") as psg_pool,
            tc.tile_pool(name="acc", bufs=1, space="PSUM") as acc_pool,
        ):
            wt_sb = consts.tile([S, D], F16)
            nc.sync.dma_start(wt_sb[:], wt_d)
            wg_sb = consts.tile([S, D], F16)
            nc.sync.dma_start(wg_sb[:], wg_d)
            bgb_sb = consts.tile([P, CHUNK], F32)
            nc.sync.dma_start(bgb_sb[:], bgb_d)

            # One-time const touches: absorb the const-DMA semaphores into
            # each engine's observed clock so hot-loop instructions never
            # need a second (DMA) wait slot.
            if OPTS["touches"]:
                dve_scratch = scratch.tile([1, 2], F32)
                nc.vector.tensor_copy(out=dve_scratch[:1, :1], in_=bgb_sb[:1, :1])
                nc.tensor.ldweights(wt_sb[:, :1])
                nc.tensor.ldweights(wg_sb[:, :1])

            pool1 = acc_pool.tile([B, D], F32)
            pool2 = acc_pool.tile([B, D], F32)

            for c in range(N_CHUNKS):
                nod = nodes_pool.tile([P, CHUNK], F16)
                nc.sync.dma_start(nod[:], nodesT[:, c * CHUNK : (c + 1) * CHUNK])
                mk = masks_pool.tile([P, TILES_PER_CHUNK, B], F16)
                nc.sync.dma_start(
                    mk[:],
                    masksT[:, c * TILES_PER_CHUNK : (c + 1) * TILES_PER_CHUNK, :],
                )

                psum_d = psd_pool.tile([P, CHUNK], F32)
                psum_g = psg_pool.tile([P, CHUNK], F32)
                for t in range(TILES_PER_CHUNK):
                    sl = bass.ts(t, P)
                    nc.tensor.matmul(
                        psum_d[:, sl], nod[:, sl], wt_sb[:], start=True, stop=True
                    )
                    nc.tensor.matmul(
                        psum_g[:, sl], nod[:, sl], wg_sb[:], start=True, stop=True
                    )

                gpre = gpre_pool.tile([P, CHUNK], F16)
                nc.vector.tensor_add(out=gpre[:], in0=psum_g[:], in1=bgb_sb[:])

                # copy_d before sigmoid: PE instructions that must observe
                # "chunk c's ACT work done" wait once on the later sigmoid
                # tick, which then also covers copy_d.
                d_t = d_pool.tile([P, TILES_PER_CHUNK, D], F16)
                nc.scalar.copy(
                    d_t[:], psum_d.rearrange("p (t d) -> p t d", d=D)
                )
                g_t = g_pool.tile([P, TILES_PER_CHUNK, D], F16)
                nc.scalar.activation(
                    g_t[:],
                    gpre.rearrange("p (t d) -> p t d", d=D),
                    mybir.ActivationFunctionType.Sigmoid
                    if OPTS["sigmoid"]
                    else mybir.ActivationFunctionType.Copy,
                )
                nc.vector.tensor_mul(out=d_t[:], in0=d_t[:], in1=g_t[:])

                if OPTS["mm2"]:
                    for t in range(TILES_PER_CHUNK):
                        first = c == 0 and t == 0
                        last = c == N_CHUNKS - 1 and t == TILES_PER_CHUNK - 1
                        nc.tensor.matmul(
                            pool1[:],
                            mk[:, t, :],
                            d_t[:, t, :],
                            start=first,
                            stop=last,
                            skip_group_check=True,
                        )
                        nc.tensor.matmul(
                            pool2[:],
                            mk[:, t, :],
                            g_t[:, t, :],
                            start=first,
                            stop=last,
                            skip_group_check=True,
                        )
                else:
                    nc.vector.tensor_copy(
                        out=dve_scratch[:1, :2], in_=d_t[:1, 0, :2]
                    )

            res = out_pool.tile([B, 2 * D], F32)
            nc.scalar.copy(res[:, :D], pool1[:])
            nc.scalar.copy(res[:, D:], pool2[:])
            nc.sync.dma_start(out_d, res[:])

    return nc


_CACHE: dict = {}


def _get_bass() -> bass.Bass:
    if "nc" not in _CACHE:
        _CACHE["nc"] = build_bass()
    return _CACHE["nc"]


def _prepare_in_maps(nodes, owner_masks, Wt, bt, Wg, bg):
    nodes_h = np.asarray(nodes, dtype=NP_F16)
    masks = np.asarray(owner_masks)
    wt_h = np.ascontiguousarray(np.asarray(Wt, dtype=NP_F16))
    wg_h = np.ascontiguousarray(np.asarray(Wg, dtype=NP_F16))
    bg32 = np.asarray(bg, dtype=np.float32)
    bgb = np.ascontiguousarray(
        np.tile(bg32[None, :], (P, CHUNK // D)).reshape(P, CHUNK)
    )

    in_maps = []
    for core in range(N_CORES):
        off = core * N_PER_CORE
        ncr = np.zeros((P, N_PAD), dtype=NP_F16)
        ncr[:, :N_PER_CORE] = nodes_h[off : off + N_PER_CORE].T
        mp = np.zeros((B, N_PAD), dtype=NP_F16)
        mp[:, :N_PER_CORE] = masks[:, off : off + N_PER_CORE]
        mkt = np.ascontiguousarray(mp.reshape(B, N_TILES, P).transpose(2, 1, 0))
        in_maps.append(
            {
                "nodesT": ncr,
                "masksT": mkt,
                "wt": wt_h,
                "wg": wg_h,
                "bgb": bgb,
            }
        )
    return in_maps


def run(inputs: dict, trace: bool = False):
    """Run the kernel. Returns (pooled [B, D] float32, BassKernelResults)."""
    nc = _get_bass()
    in_maps = _prepare_in_maps(**inputs)
    rb = run_bass_kernel_spmd(
        nc, in_maps, core_ids=list(range(N_CORES)), trace=trace
    )
    parts = np.stack([r["out"].astype(np.float64) for r in rb.results])
    tot = parts.sum(axis=0)
    bt64 = np.asarray(inputs["bt"], dtype=np.float64)
    pooled = tot[:, :D] + tot[:, D:] * bt64[None, :]
    return pooled.astype(np.float32), rb


def kernel(**inputs) -> np.ndarray:
    out, _ = run(inputs, trace=False)
    return out


if __name__ == "__main__":
    rng = np.random.default_rng(0)
    demo = {
        "nodes": rng.standard_normal((N_TOTAL, S), dtype=np.float32),
        "owner_masks": rng.integers(0, 2, (B, N_TOTAL)).astype(np.int32),
        "Wt": rng.standard_normal((S, D), dtype=np.float32) * 0.09,
        "bt": rng.standard_normal(D).astype(np.float32) * 0.09,
        "Wg": rng.standard_normal((S, D), dtype=np.float32) * 0.09,
        "bg": rng.standard_normal(D).astype(np.float32) * 0.09,
    }
    out = kernel(**demo)
    print(out.shape, out.dtype, np.abs(out).mean())


# revision 11
# speedup vs baseline: 1.1872x; 1.1872x over previous
"""Trainium2 Bass kernel for nn_Aggregator (gnn_message_passing).

pooled[B,D] = owner_masks.f32 @ ((nodes@Wt + bt) * sigmoid(nodes@Wg + bg))

Sharding: nodes (and owner_masks columns) split along N across 8 cores.
Each core computes a partial [B, 2D] = [M@(A*G) | M@G]; the host sums the
8 partials and applies the bt column bias algebraically:
    pooled = sum_c pool1_c + (sum_c pool2_c) * bt[None, :]
(exact: (A + 1 bt^T) * G = A*G + (1 bt^T)*G and M @ ((1 bt^T)*G) =
(M@G) diag(bt)).

Device pipeline (per core; fp16 inputs, fp32 accumulation):
  nodesT  [S=128, NPAD]   node features transposed, S on partitions
  masksT  [128, NT, B]    partition-major transposed masks:
                          masksT[p, t, b] = owner_masks[b, t*128+p]
  per chunk of 4 128-node tiles:
    PE : MM(psum_d[:,sl], lhsT=nodesT[:,sl], rhs=Wt)  (A tile, [n,D])
         MM(psum_g[:,sl], lhsT=nodesT[:,sl], rhs=Wg)  (G_pre,  [n,D])
    DVE: gpre = psum_g + bg_bcast          (fp16 out)
    ACT: mg[:, :, D:]  = sigmoid(gpre)     (fp16)
    ACT: mg[:, :, :D]  = copy(psum_d)      (fp16)
    DVE: mg[:, :, :D] *= mg[:, :, D:]      (in-place msg = A*G)
    PE : pool12[B, 2D] += masksT_t.T @ mg_t   (one 256-wide MM per tile)
"""

import json

import numpy as np

import concourse.bass as bass
import concourse.mybir as mybir
import concourse.tile as tile
from concourse import bass2jax as _b2j
from concourse import bass_utils as _bu
from concourse.bass_utils import run_bass_kernel_spmd


def _split_excess_waits_json(bir_json) -> bytes:
    """Walrus in this container accepts at most 1 embedded sem-wait per
    instruction (2 for EventSemaphore). Tile emits instructions (notably the
    kernel-tail Drain) with more. Move excess waits onto injected
    EventSemaphore instructions placed immediately before the offender in
    the same engine stream — identical blocking semantics."""
    if isinstance(bir_json, str):
        bir_json = bir_json.encode()
    d = json.loads(bir_json)
    counter = [0]

    def fix_block(b):
        new = []
        for inst in b.get("instructions", []):
            si = inst.get("sync_info")
            waits = (si or {}).get("on_wait") or []
            cap = 2 if inst.get("opcode") == "EventSemaphore" else 1
            if len(waits) > cap:
                keep, excess = waits[:cap], waits[cap:]
                for j in range(0, len(excess), 2):
                    counter[0] += 1
                    new.append(
                        {
                            "debug": inst.get("debug"),
                            "engine": inst["engine"],
                            "ins": [],
                            "outs": [],
                            "name": f"antsplit_ev_{counter[0]}",
                            "opcode": "EventSemaphore",
                            "sync_info": {
                                "on_update": [],
                                "on_wait": excess[j : j + 2],
                            },
                        }
                    )
                si["on_wait"] = keep
            new.append(inst)
        b["instructions"] = new
        for sb in b.get("blocks", []):
            fix_block(sb)

    for f in d.get("functions", []):
        for blk in f.get("blocks", []):
            fix_block(blk)
    return json.dumps(d).encode()


if not getattr(_bu, "_ant_split_waits_patched", False):
    _orig_compile_bir_kernel = _bu.compile_bir_kernel

    def _patched_compile_bir_kernel(bir_json, tmpdir, neff_name="file.neff"):
        return _orig_compile_bir_kernel(
            _split_excess_waits_json(bir_json), tmpdir, neff_name
        )

    _bu.compile_bir_kernel = _patched_compile_bir_kernel
    _b2j.compile_bir_kernel = _patched_compile_bir_kernel
    _bu._ant_split_waits_patched = True

N_CORES = 8
N_TOTAL = 500_000
B = 128
S = 128
D = 128
P = 128

N_PER_CORE = N_TOTAL // N_CORES          # 62500
TILES_PER_CHUNK = 4
CHUNK = TILES_PER_CHUNK * P              # 512
N_CHUNKS = -(-N_PER_CORE // CHUNK)       # 123
N_TILES = N_CHUNKS * TILES_PER_CHUNK     # 492
N_PAD = N_TILES * P                      # 62976

F16 = mybir.dt.float16
F32 = mybir.dt.float32
NP_F16 = np.float16


# bisect/debug switches
OPTS = {
    "touches": True,     # one-time const touch ops
    "sigmoid": True,     # False -> plain Copy instead of Sigmoid
    "mm2": True,         # False -> skip the pooling matmuls (evict psum_d instead)
    "mm1": True,         # False -> skip the feature matmuls entirely
    "elemwise": True,    # False -> skip DVE/ACT elementwise ops
}


def build_bass() -> bass.Bass:
    nc = bass.Bass()

    nodesT = nc.dram_tensor("nodesT", [P, N_PAD], F16, kind="ExternalInput").ap()
    masksT = nc.dram_tensor("masksT", [P, N_TILES, B], F16, kind="ExternalInput").ap()
    wt_d = nc.dram_tensor("wt", [S, D], F16, kind="ExternalInput").ap()
    wg_d = nc.dram_tensor("wg", [S, D], F16, kind="ExternalInput").ap()
    bgb_d = nc.dram_tensor("bgb", [P, CHUNK], F32, kind="ExternalInput").ap()
    out_d = nc.dram_tensor("out", [B, 2 * D], F32, kind="ExternalOutput").ap()

    with tile.TileContext(nc) as tc:
        with (
            tc.tile_pool(name="consts", bufs=1) as consts,
            tc.tile_pool(name="scratch", bufs=1) as scratch,
            tc.tile_pool(name="nodes", bufs=4) as nodes_pool,
            tc.tile_pool(name="masks", bufs=4) as masks_pool,
            tc.tile_pool(name="gpre", bufs=3) as gpre_pool,
            tc.tile_pool(name="dt", bufs=3) as d_pool,
            tc.tile_pool(name="gt", bufs=3) as g_pool,
            tc.tile_pool(name="outs", bufs=1) as out_pool,
            tc.tile_pool(name="psd", bufs=2, space="PSUM") as psd_pool,
            tc.tile_pool(name="psg", bufs=2, space="PSUM") as psg_pool,
            tc.tile_pool(name="acc", bufs=1, space="PSUM") as acc_pool,
        ):
            wt_sb = consts.tile([S, D], F16)
            nc.sync.dma_start(wt_sb[:], wt_d)
            wg_sb = consts.tile([S, D], F16)
            nc.sync.dma_start(wg_sb[:], wg_d)
            bgb_sb = consts.tile([P, CHUNK], F32)
            nc.sync.dma_start(bgb_sb[:], bgb_d)

            # One-time const touches: absorb the const-DMA semaphores into
            # each engine's observed clock so hot-loop instructions never
            # need a second (DMA) wait slot.
            if OPTS["touches"]:
                dve_scratch = scratch.tile([1, 2], F32)
                nc.vector.tensor_copy(out=dve_scratch[:1, :1], in_=bgb_sb[:1, :1])
                nc.tensor.ldweights(wt_sb[:, :1])
                nc.tensor.ldweights(wg_sb[:, :1])

            if OPTS["mm2"]:
                pool1 = acc_pool.tile([B, D], F32)
                pool2 = acc_pool.tile([B, D], F32)

            for c in range(N_CHUNKS):
                nod = nodes_pool.tile([P, CHUNK], F16)
                nc.sync.dma_start(nod[:], nodesT[:, c * CHUNK : (c + 1) * CHUNK])
                mk = masks_pool.tile([P, TILES_PER_CHUNK, B], F16)
                nc.sync.dma_start(
                    mk[:],
                    masksT[:, c * TILES_PER_CHUNK : (c + 1) * TILES_PER_CHUNK, :],
                )

                if not OPTS["mm1"]:
                    # DMA-only mode: keep the loads live with 1-elem reads
                    nc.vector.tensor_copy(out=dve_scratch[:1, :1], in_=nod[:1, :1])
                    nc.vector.tensor_copy(
                        out=dve_scratch[:1, 1:2], in_=mk[:1, 0, :1]
                    )
                    continue

                psum_d = psd_pool.tile([P, CHUNK], F32)
                psum_g = psg_pool.tile([P, CHUNK], F32)
                for t in range(TILES_PER_CHUNK):
                    sl = bass.ts(t, P)
                    nc.tensor.matmul(
                        psum_d[:, sl], nod[:, sl], wt_sb[:], start=True, stop=True
                    )
                    nc.tensor.matmul(
                        psum_g[:, sl], nod[:, sl], wg_sb[:], start=True, stop=True
                    )

                if not OPTS["elemwise"]:
                    nc.vector.tensor_copy(out=dve_scratch[:1, :1], in_=psum_d[:1, :1])
                    nc.vector.tensor_copy(
                        out=dve_scratch[:1, 1:2], in_=psum_g[:1, :1]
                    )
                    nc.vector.tensor_copy(
                        out=dve_scratch[:1, 1:2], in_=mk[:1, 0, :1]
                    )
                    continue

                gpre = gpre_pool.tile([P, CHUNK], F16)
                nc.vector.tensor_add(out=gpre[:], in0=psum_g[:], in1=bgb_sb[:])

                # copy_d before sigmoid: PE instructions that must observe
                # "chunk c's ACT work done" wait once on the later sigmoid
                # tick, which then also covers copy_d.
                d_t = d_pool.tile([P, TILES_PER_CHUNK, D], F16)
                nc.scalar.copy(
                    d_t[:], psum_d.rearrange("p (t d) -> p t d", d=D)
                )
                g_t = g_pool.tile([P, TILES_PER_CHUNK, D], F16)
                nc.scalar.activation(
                    g_t[:],
                    gpre.rearrange("p (t d) -> p t d", d=D),
                    mybir.ActivationFunctionType.Sigmoid
                    if OPTS["sigmoid"]
                    else mybir.ActivationFunctionType.Copy,
                )
                nc.vector.tensor_mul(out=d_t[:], in0=d_t[:], in1=g_t[:])

                if OPTS["mm2"]:
                    for t in range(TILES_PER_CHUNK):
                        first = c == 0 and t == 0
                        last = c == N_CHUNKS - 1 and t == TILES_PER_CHUNK - 1
                        nc.tensor.matmul(
                            pool1[:],
                            mk[:, t, :],
                            d_t[:, t, :],
                            start=first,
                            stop=last,
                            skip_group_check=True,
                        )
                        nc.tensor.matmul(
                            pool2[:],
                            mk[:, t, :],
                            g_t[:, t, :],
                            start=first,
                            stop=last,
                            skip_group_check=True,
                        )
                else:
                    nc.vector.tensor_copy(
                        out=dve_scratch[:1, :2], in_=d_t[:1, 0, :2]
                    )

            res = out_pool.tile([B, 2 * D], F32)
            if OPTS["mm2"]:
                nc.scalar.copy(res[:, :D], pool1[:])
                nc.scalar.copy(res[:, D:], pool2[:])
            else:
                nc.vector.tensor_copy(out=res[:1, :2], in_=dve_scratch[:1, :2])
            nc.sync.dma_start(out_d, res[:])

    return nc


_CACHE: dict = {}


def _get_bass() -> bass.Bass:
    if "nc" not in _CACHE:
        _CACHE["nc"] = build_bass()
    return _CACHE["nc"]


def _prepare_in_maps(nodes, owner_masks, Wt, bt, Wg, bg):
    nodes_h = np.asarray(nodes, dtype=NP_F16)
    masks = np.asarray(owner_masks)
    wt_h = np.ascontiguousarray(np.asarray(Wt, dtype=NP_F16))
    wg_h = np.ascontiguousarray(np.asarray(Wg, dtype=NP_F16))
    bg32 = np.asarray(bg, dtype=np.float32)
    bgb = np.ascontiguousarray(
        np.tile(bg32[None, :], (P, CHUNK // D)).reshape(P, CHUNK)
    )

    in_maps = []
    for core in range(N_CORES):
        off = core * N_PER_CORE
        ncr = np.zeros((P, N_PAD), dtype=NP_F16)
        ncr[:, :N_PER_CORE] = nodes_h[off : off + N_PER_CORE].T
        mp = np.zeros((B, N_PAD), dtype=NP_F16)
        mp[:, :N_PER_CORE] = masks[:, off : off + N_PER_CORE]
        mkt = np.ascontiguousarray(mp.reshape(B, N_TILES, P).transpose(2, 1, 0))
        in_maps.append(
            {
                "nodesT": ncr,
                "masksT": mkt,
                "wt": wt_h,
                "wg": wg_h,
                "bgb": bgb,
            }
        )
    return in_maps


def run(inputs: dict, trace: bool = False):
    """Run the kernel. Returns (pooled [B, D] float32, BassKernelResults)."""
    nc = _get_bass()
    in_maps = _prepare_in_maps(**inputs)
    rb = run_bass_kernel_spmd(
        nc, in_maps, core_ids=list(range(N_CORES)), trace=trace
    )
    parts = np.stack([r["out"].astype(np.float64) for r in rb.results])
    tot = parts.sum(axis=0)
    bt64 = np.asarray(inputs["bt"], dtype=np.float64)
    pooled = tot[:, :D] + tot[:, D:] * bt64[None, :]
    return pooled.astype(np.float32), rb


def kernel(**inputs) -> np.ndarray:
    out, _ = run(inputs, trace=False)
    return out


if __name__ == "__main__":
    rng = np.random.default_rng(0)
    demo = {
        "nodes": rng.standard_normal((N_TOTAL, S), dtype=np.float32),
        "owner_masks": rng.integers(0, 2, (B, N_TOTAL)).astype(np.int32),
        "Wt": rng.standard_normal((S, D), dtype=np.float32) * 0.09,
        "bt": rng.standard_normal(D).astype(np.float32) * 0.09,
        "Wg": rng.standard_normal((S, D), dtype=np.float32) * 0.09,
        "bg": rng.standard_normal(D).astype(np.float32) * 0.09,
    }
    out = kernel(**demo)
    print(out.shape, out.dtype, np.abs(out).mean())
